# revision 11
# baseline (speedup 1.0000x reference)
"""Multi-head attention (B=4, S=2048, D=1024, H=16) on 8 TRN2 NeuronCores.

Sharding: core c <- batch c//2, heads 8*(c%2) .. 8*(c%2)+8 (Megatron-style:
Wq/Wk/Wv column-parallel, Wo row-parallel). No collectives: the two partial
outputs per batch are summed on the host (plus the bo bias).

Per-core kernel strategy (v6 — bf16, flat depth-2 attention pipeline):
  - All matmul operands are bf16 (PSUM accumulation stays fp32). The PE
    moving-operand SBUF port supplies 2 B/lane/cycle at 2.4 GHz, so fp32
    streams are capped at an effective 1.2 GHz while bf16 streams run at
    full clock: bf16 halves every matmul's wall time.
  - q^T, k^T computed directly in [head_dim, seq] layout (out = W^T.T @ X^T),
    v computed in natural [seq, head_dim] layout with a ones column appended.
  - Scores computed transposed: ST[s_k, s_q] = k . q, so softmax exp is pure
    elementwise (no max subtraction needed: scores ~ N(0,1) after 1/8 scale)
    and no on-chip transposes are needed anywhere.
  - ctx^T[c, s_q] accumulated as v_aug^T @ exp(ST); the ones column yields the
    softmax denominator l[s_q] as psum row 64 for free.
  - Normalization by 1/l: copy the l row to partition 0 (reciprocal_approx_fast
    misreads partition-offset APs), approx-reciprocal, gpsimd broadcast, mul.
  - The 256 attention iterations run as ONE flat software pipeline with
    explicit depth 2: iteration i emits scores(i), exp(i-1), ctx(i-2). The
    exp's input is a full iteration old, so ACT never eats the ~100 ns
    cross-engine semaphore latency, and the ~300 ns/iter of PE slack
    accumulates into real lookahead.
  - Only v and the jt=0 q/k projections run before attention; remaining q/k
    projections drop into the pipeline as PE filler (2 chunks per 16-iter
    block), absorbed by the ACT shadow. Blocks run jt-major
    [(2jt,s0),(2jt+1,s0),(2jt,s1),(2jt+1,s1)] so every head's sqb=0 finishes
    by block 13 and the first half of the output projection overlaps the
    last two blocks. PSUM: st x2 (4 banks) + ctx0/ctx1 (2) + proj x2 (2) = 8.
"""
import sys

sys.path.insert(0, "/opt/trn_rl_repo")
import numpy as np
import ml_dtypes

import concourse.bass as bass
import concourse.bacc as bacc
import concourse.mybir as mybir
import concourse.tile as tile
from concourse.bass_utils import run_bass_kernel_spmd

f32 = mybir.dt.float32
bf16 = mybir.dt.bfloat16
EXP = mybir.ActivationFunctionType.Exp

S = 2048          # sequence length
D = 1024          # model dim
HC = 8            # heads per core
DK = 64           # head dim
JC = HC * DK      # per-core projection width (512)
SCALE = 0.125     # 1/sqrt(DK)
N_CORES = 8

# Block order: jt-major so head-pair jt's q/k projections are only needed
# 4 blocks after head-pair jt-1's blocks start, and all sqb=0 blocks are
# done by index 13.
BLOCKS = []
for _jt in range(4):
    BLOCKS += [(2 * _jt, 0), (2 * _jt + 1, 0), (2 * _jt, 1), (2 * _jt + 1, 1)]


def _stage12(nc, tc, work, io, sb):
    """v + jt0 q/k projections, then the flat attention pipeline with the
    remaining projections and the first half of the output projection
    interleaved as PE filler."""
    nc.vector.memset(sb.v_sb[:, :, :, DK], 1.0)
    with (
        tc.tile_pool(name="w1", bufs=1) as wp,
        tc.tile_pool(name="x1", bufs=1) as xp,
        tc.tile_pool(name="psproj", bufs=2, space="PSUM") as pp,
        tc.tile_pool(name="ps2st", bufs=2, space="PSUM") as pp_st,
        tc.tile_pool(name="ps2ctx", bufs=1, space="PSUM") as pp_ctx,
        tc.tile_pool(name="att", bufs=4) as att,
        tc.tile_pool(name="att2", bufs=1) as att2,
    ):
        wq_sb = wp.tile([128, 8, JC], bf16, tag="wq")
        wk_sb = wp.tile([128, 8, JC], bf16, tag="wk")
        wv_sb = wp.tile([128, 8, JC], bf16, tag="wv")
        # per-chunk x tiles so a projection chunk only waits on its own DMA
        xv = [xp.tile([128, 8, 512], bf16, tag=f"xvk{c}", name=f"xv{c}")
              for c in range(4)]
        xq = [xp.tile([128, 8, 512], bf16, tag=f"xq{c}", name=f"xq{c}")
              for c in range(4)]
        # k chunks reuse the v chunk slots (v is fully consumed before k DMAs)
        xk = [xp.tile([128, 8, 512], bf16, tag=f"xvk{c}", name=f"xk{c}")
              for c in range(4)]
        nc.sync.dma_start(wv_sb[:], io.wvt.rearrange("(kt p) j -> p kt j", p=128))
        for ch in range(4):
            nc.sync.dma_start(
                xv[ch][:],
                io.vt[:, ch * 512:(ch + 1) * 512].rearrange(
                    "(kt p) s -> p kt s", p=128),
            )
        nc.sync.dma_start(wq_sb[:], io.wqt.rearrange("(kt p) j -> p kt j", p=128))
        nc.sync.dma_start(wk_sb[:], io.wkt.rearrange("(kt p) j -> p kt j", p=128))
        for ch in range(4):
            nc.sync.dma_start(
                xq[ch][:],
                io.qt[:, ch * 512:(ch + 1) * 512].rearrange(
                    "(kt p) s -> p kt s", p=128),
            )
        for ch in range(4):
            nc.sync.dma_start(
                xk[ch][:],
                io.kt[:, ch * 512:(ch + 1) * 512].rearrange(
                    "(kt p) s -> p kt s", p=128),
            )

        # v projection (all of it is needed before the first ctx matmul).
        for st_i in range(16):
            ps = pp.tile([128, JC], f32, tag="proj")
            for ktile in range(8):
                nc.tensor.matmul(
                    ps[:],
                    xv[st_i // 4][:, ktile, (st_i % 4) * 128:(st_i % 4 + 1) * 128],
                    wv_sb[:, ktile, :],
                    start=(ktile == 0),
                    stop=(ktile == 7),
                )
            nc.vector.tensor_add(
                sb.v_sb[:, st_i, :, 0:DK],
                ps[:].rearrange("p (h c) -> p h c", h=HC),
                sb.bvb_sb[:].rearrange("p (h c) -> p h c", h=HC),
            )

        def proj_chunk(which, jt, sc):
            """One [128,512] projection chunk: q or k, head-pair jt, seq chunk sc."""
            x_sb, w_sb, o_sb, b_sb = (
                (xq[sc], wq_sb, sb.qT_sb, sb.bq_sb) if which == "q"
                else (xk[sc], wk_sb, sb.kT_sb, sb.bk_sb))
            ps = pp.tile([128, 512], f32, tag="proj")
            for ktile in range(8):
                nc.tensor.matmul(
                    ps[:],
                    w_sb[:, ktile, jt * 128:(jt + 1) * 128],
                    x_sb[:, ktile, :],
                    start=(ktile == 0),
                    stop=(ktile == 7),
                )
            nc.vector.tensor_scalar_add(
                o_sb[:, jt, sc * 512:(sc + 1) * 512], ps[:], b_sb[:, jt:jt + 1])

        for sc in range(4):
            proj_chunk("q", 0, sc)
        for sc in range(4):
            proj_chunk("k", 0, sc)

        # Fillers: head-pair jt's 8 projection chunks spread 2-per-block over
        # the 4 blocks of head-pair jt-1 (at iterations 5 and 13).
        fillers = {}  # (block, k) -> (which, jt, sc)
        for jt in range(1, 4):
            chunks = [("q", jt, sc) for sc in range(4)] + \
                     [("k", jt, sc) for sc in range(4)]
            base = (jt - 1) * 4
            for i, c in enumerate(chunks):
                fillers[(base + i // 2, 5 if i % 2 == 0 else 13)] = c

        def emit_scores(h, sqb, k, st):
            lhs = sb.kT_sb[64 * (h % 2):64 * (h % 2) + 64,
                           h // 2, k * 128:(k + 1) * 128]
            q0 = sqb * 1024
            nc.tensor.matmul(
                st[:, 0:512], lhs,
                sb.qT_sb[64 * (h % 2):64 * (h % 2) + 64, h // 2, q0:q0 + 512],
                start=True, stop=True,
            )
            nc.tensor.matmul(
                st[:, 512:1024], lhs,
                sb.qT_sb[64 * (h % 2):64 * (h % 2) + 64, h // 2,
                         q0 + 512:q0 + 1024],
                start=True, stop=True,
            )

        ctxs = {}

        def emit_ctx(h, sqb, k, pt):
            c0, c1 = ctxs[(h, sqb)]
            vt = sb.v_sb[:, k, h, :]
            nc.tensor.matmul(c0[:], vt, pt[:, 0:512], start=(k == 0), stop=(k == 15))
            nc.tensor.matmul(c1[:], vt, pt[:, 512:1024], start=(k == 0), stop=(k == 15))
            if k == 15:
                jt = h // 2
                pbase = 64 * (h % 2)
                for ci, ctx in enumerate((c0, c1)):
                    sq = sqb * 2 + ci
                    lr = att2.tile([1, 512], f32, tag=f"l{ci}", name=f"l_{h}_{sq}")
                    nc.vector.tensor_copy(lr[:], ctx[DK:DK + 1, :])
                    r = att2.tile([1, 512], f32, tag=f"r{ci}", name=f"r_{h}_{sq}")
                    nc.vector.reciprocal_approx_fast(r[:], lr[:])
                    rb = att2.tile([64, 512], f32, tag=f"rb{ci}", name=f"rb_{h}_{sq}")
                    nc.gpsimd.partition_broadcast(rb[:], r[:])
                    nc.vector.tensor_mul(
                        sb.ctxn_sb[pbase:pbase + 64, jt, sq * 512:(sq + 1) * 512],
                        ctx[0:DK, :], rb[:],
                    )
                del ctxs[(h, sqb)]

        def out_proj(sq2, n):
            """One [128,512] output-projection group + copy-out + DMA."""
            ps = pp.tile([128, 512], f32, tag="proj")
            for p in range(4):
                nc.tensor.matmul(
                    ps[:],
                    sb.ctxn_sb[:, p, sq2 * 128:(sq2 + 1) * 128],
                    sb.wot_sb[:, p, n * 512:(n + 1) * 512],
                    start=(p == 0), stop=(p == 3),
                )
            ob = work.tile([128, 512], f32, tag="ob")
            nc.vector.tensor_copy(ob[:], ps[:])
            nc.sync.dma_start(
                io.out[sq2 * 128:(sq2 + 1) * 128, n * 512:(n + 1) * 512],
                ob[:],
            )

        # Flat depth-2 pipeline over all 256 (block, k) iterations:
        # iteration i emits scores(i), exp(i-1), ctx(i-2).
        iters = [(h, sqb, k) for (h, sqb) in BLOCKS for k in range(16)]
        exps = {}   # i -> (h, sqb, k, st)
        pts = {}    # i -> (h, sqb, k, pt)
        for i, (h, sqb, k) in enumerate(iters):
            if k == 0:
                ctxs[(h, sqb)] = (
                    pp_ctx.tile([DK + 1, 512], f32, tag="ctx0",
                                name=f"ctx0_{h}_{sqb}"),
                    pp_ctx.tile([DK + 1, 512], f32, tag="ctx1",
                                name=f"ctx1_{h}_{sqb}"),
                )
            st = pp_st.tile([128, 1024], f32, tag="st")
            emit_scores(h, sqb, k, st)
            exps[i] = (h, sqb, k, st)
            if i - 1 in exps:
                eh, esqb, ek, est = exps.pop(i - 1)
                pt = att.tile([128, 1024], bf16, tag="pt")
                nc.scalar.activation(pt[:], est[:], EXP, scale=SCALE)
                pts[i - 1] = (eh, esqb, ek, pt)
            if i - 2 in pts:
                emit_ctx(*pts.pop(i - 2))
            f = fillers.get((i // 16, i % 16))
            if f is not None:
                proj_chunk(*f)
            # first half of the output projection rides in the last 2 blocks
            if i == 14 * 16 + 8:
                for sq2 in range(4):
                    out_proj(sq2, 0)
                    out_proj(sq2, 1)
            if i == 15 * 16 + 4:
                for sq2 in range(4, 8):
                    out_proj(sq2, 0)
                    out_proj(sq2, 1)
        # drain the pipeline
        for i in (256, 257):
            if i - 1 in exps:
                eh, esqb, ek, est = exps.pop(i - 1)
                pt = att.tile([128, 1024], bf16, tag="pt")
                nc.scalar.activation(pt[:], est[:], EXP, scale=SCALE)
                pts[i - 1] = (eh, esqb, ek, pt)
            if i - 2 in pts:
                emit_ctx(*pts.pop(i - 2))
        # second half of the output projection
        for sq2 in range(8, 16):
            out_proj(sq2, 0)
            out_proj(sq2, 1)


class _NS:
    pass


def build_nc(repeats=1):
    nc = bacc.Bacc(None, target_bir_lowering=False, debug=False)

    io = _NS()
    io.qt = nc.dram_tensor("qt", [D, S], bf16, kind="ExternalInput")
    io.kt = nc.dram_tensor("kt", [D, S], bf16, kind="ExternalInput")
    io.vt = nc.dram_tensor("vt", [D, S], bf16, kind="ExternalInput")
    io.wqt = nc.dram_tensor("wqt", [D, JC], bf16, kind="ExternalInput")
    io.wkt = nc.dram_tensor("wkt", [D, JC], bf16, kind="ExternalInput")
    io.wvt = nc.dram_tensor("wvt", [D, JC], bf16, kind="ExternalInput")
    io.wot = nc.dram_tensor("wot", [JC, D], bf16, kind="ExternalInput")
    io.bq = nc.dram_tensor("bq", [128, 4], f32, kind="ExternalInput")
    io.bk = nc.dram_tensor("bk", [128, 4], f32, kind="ExternalInput")
    io.bvb = nc.dram_tensor("bvb", [128, JC], f32, kind="ExternalInput")
    io.out = nc.dram_tensor("out", [S, D], f32, kind="ExternalOutput")

    with tile.TileContext(nc) as tc:
        for _rep in range(repeats):
            with (
                tc.tile_pool(name="big", bufs=1) as big,
                tc.tile_pool(name="work", bufs=3) as work,
            ):
                sb = _NS()
                sb.qT_sb = big.tile([128, 4, S], bf16)           # [p, jt, s]
                sb.kT_sb = big.tile([128, 4, S], bf16)
                sb.v_sb = big.tile([128, 16, HC, DK + 1], bf16)  # [p, st, h, c]
                sb.bq_sb = big.tile([128, 4], f32)
                sb.bk_sb = big.tile([128, 4], f32)
                sb.bvb_sb = big.tile([128, JC], f32)

                nc.sync.dma_start(sb.bq_sb[:], io.bq[:])
                nc.sync.dma_start(sb.bk_sb[:], io.bk[:])
                nc.sync.dma_start(sb.bvb_sb[:], io.bvb[:])

                with tc.tile_pool(name="big2", bufs=1) as big2:
                    sb.ctxn_sb = big2.tile([128, 4, S], bf16)    # [p, pair, s]
                    sb.wot_sb = big2.tile([128, 4, D], bf16)
                    nc.sync.dma_start(
                        sb.wot_sb[:],
                        io.wot.rearrange("(kt p) j -> p kt j", p=128),
                    )
                    _stage12(nc, tc, work, io, sb)

    nc.compile()
    return nc


_NC = None


def _get_nc():
    global _NC
    if _NC is None:
        _NC = build_nc()
    return _NC


def make_in_maps(Q, K, V, Wq, bq, Wk, bk, Wv, bv, Wo, bo):
    asb = lambda x: np.ascontiguousarray(
        np.asarray(x, dtype=np.float32).astype(ml_dtypes.bfloat16))
    asf = lambda x: np.ascontiguousarray(np.asarray(x, dtype=np.float32))
    in_maps = []
    for c in range(N_CORES):
        b = c // 2
        j0 = JC * (c % 2)
        jsl = slice(j0, j0 + JC)
        in_maps.append({
            "qt": asb(np.asarray(Q)[b].T),
            "kt": asb(np.asarray(K)[b].T),
            "vt": asb(np.asarray(V)[b].T),
            "wqt": asb(np.asarray(Wq)[jsl].T),
            "wkt": asb(np.asarray(Wk)[jsl].T),
            "wvt": asb(np.asarray(Wv)[jsl].T),
            "wot": asb(np.asarray(Wo)[:, jsl].T),
            "bq": asf(np.asarray(bq)[jsl].reshape(4, 128).T),
            "bk": asf(np.asarray(bk)[jsl].reshape(4, 128).T),
            "bvb": asf(np.broadcast_to(np.asarray(bv)[jsl], (128, JC))),
        })
    return in_maps


def kernel(Q, K, V, Wq, bq, Wk, bk, Wv, bv, Wo, bo, _trace=False, _trace_kwargs=None):
    nc = _get_nc()
    in_maps = make_in_maps(Q, K, V, Wq, bq, Wk, bk, Wv, bv, Wo, bo)
    res = run_bass_kernel_spmd(
        nc, in_maps, core_ids=list(range(N_CORES)),
        trace=_trace, **(_trace_kwargs or {}),
    )
    parts = [res.results[c]["out"] for c in range(N_CORES)]
    bo_np = np.asarray(bo, dtype=np.float32)
    O = np.stack([parts[2 * b] + parts[2 * b + 1] + bo_np for b in range(4)])
    kernel.last_results = res
    return O.astype(np.float32)


# revision 14
# speedup vs baseline: 1.0889x; 1.0889x over previous
"""Multi-head attention (B=4, S=2048, D=1024, H=16) on 8 TRN2 NeuronCores.

Sharding: core c <- batch c//2, heads 8*(c%2) .. 8*(c%2)+8 (Megatron-style:
Wq/Wk/Wv column-parallel, Wo row-parallel). No collectives: the two partial
outputs per batch are summed on the host (plus the bo bias).

Per-core kernel strategy (v6 — bf16, flat depth-2 attention pipeline):
  - All matmul operands are bf16 (PSUM accumulation stays fp32). The PE
    moving-operand SBUF port supplies 2 B/lane/cycle at 2.4 GHz, so fp32
    streams are capped at an effective 1.2 GHz while bf16 streams run at
    full clock: bf16 halves every matmul's wall time.
  - q^T, k^T computed directly in [head_dim, seq] layout (out = W^T.T @ X^T),
    v computed in natural [seq, head_dim] layout with a ones column appended.
  - Scores computed transposed: ST[s_k, s_q] = k . q, so softmax exp is pure
    elementwise (no max subtraction needed: scores ~ N(0,1) after 1/8 scale)
    and no on-chip transposes are needed anywhere.
  - ctx^T[c, s_q] accumulated as v_aug^T @ exp(ST); the ones column yields the
    softmax denominator l[s_q] as psum row 64 for free.
  - Normalization by 1/l: copy the l row to partition 0 (reciprocal_approx_fast
    misreads partition-offset APs), approx-reciprocal, gpsimd broadcast, mul.
  - The 256 attention iterations run as ONE flat software pipeline with
    explicit depth 2: iteration i emits scores(i), exp(i-1), ctx(i-2). The
    exp's input is a full iteration old, so ACT never eats the ~100 ns
    cross-engine semaphore latency, and the ~300 ns/iter of PE slack
    accumulates into real lookahead.
  - Only v and the jt=0 q/k projections run before attention; remaining q/k
    projections drop into the pipeline as PE filler (2 chunks per 16-iter
    block), absorbed by the ACT shadow. Blocks run jt-major
    [(2jt,s0),(2jt+1,s0),(2jt,s1),(2jt+1,s1)] so every head's sqb=0 finishes
    by block 13 and the first half of the output projection overlaps the
    last two blocks. PSUM: st x2 (4 banks) + ctx0/ctx1 (2) + proj x2 (2) = 8.
"""
import sys

sys.path.insert(0, "/opt/trn_rl_repo")
import numpy as np
import ml_dtypes

import concourse.bass as bass
import concourse.bacc as bacc
import concourse.mybir as mybir
import concourse.tile as tile
from concourse.bass_utils import run_bass_kernel_spmd

f32 = mybir.dt.float32
bf16 = mybir.dt.bfloat16
EXP = mybir.ActivationFunctionType.Exp

S = 2048          # sequence length
D = 1024          # model dim
HC = 8            # heads per core
DK = 64           # head dim
JC = HC * DK      # per-core projection width (512)
SCALE = 0.125     # 1/sqrt(DK)
N_CORES = 8

# Block order: jt-major so head-pair jt's q/k projections are only needed
# 4 blocks after head-pair jt-1's blocks start, and all sqb=0 blocks are
# done by index 13.
BLOCKS = []
for _jt in range(4):
    BLOCKS += [(2 * _jt, 0), (2 * _jt + 1, 0), (2 * _jt, 1), (2 * _jt + 1, 1)]


def _stage12(nc, tc, work, io, sb):
    """v + jt0 q/k projections, then the flat attention pipeline with the
    remaining projections and the first half of the output projection
    interleaved as PE filler."""
    nc.vector.memset(sb.v_sb[:, :, :, DK], 1.0)
    with (
        tc.tile_pool(name="w1", bufs=1) as wp,
        tc.tile_pool(name="x1", bufs=1) as xp,
        tc.tile_pool(name="psproj", bufs=2, space="PSUM") as pp,
        tc.tile_pool(name="ps2st", bufs=2, space="PSUM") as pp_st,
        tc.tile_pool(name="ps2ctx", bufs=1, space="PSUM") as pp_ctx,
        tc.tile_pool(name="att", bufs=4) as att,
        tc.tile_pool(name="att2", bufs=1) as att2,
    ):
        wq_sb = wp.tile([128, 8, JC], bf16, tag="wq")
        wk_sb = wp.tile([128, 8, JC], bf16, tag="wk")
        wv_sb = wp.tile([128, 8, JC], bf16, tag="wv")
        # per-chunk x tiles so a projection chunk only waits on its own DMA
        xv = [xp.tile([128, 8, 512], bf16, tag=f"xvk{c}", name=f"xv{c}")
              for c in range(4)]
        xq = [xp.tile([128, 8, 512], bf16, tag=f"xq{c}", name=f"xq{c}")
              for c in range(4)]
        # k chunks reuse the v chunk slots (v is fully consumed before k DMAs)
        xk = [xp.tile([128, 8, 512], bf16, tag=f"xvk{c}", name=f"xk{c}")
              for c in range(4)]
        nc.sync.dma_start(wv_sb[:], io.wvt.rearrange("(kt p) j -> p kt j", p=128))
        for ch in range(4):
            nc.sync.dma_start(
                xv[ch][:],
                io.vt[:, ch * 512:(ch + 1) * 512].rearrange(
                    "(kt p) s -> p kt s", p=128),
            )
        nc.sync.dma_start(wq_sb[:], io.wqt.rearrange("(kt p) j -> p kt j", p=128))
        nc.sync.dma_start(wk_sb[:], io.wkt.rearrange("(kt p) j -> p kt j", p=128))
        for ch in range(4):
            nc.sync.dma_start(
                xq[ch][:],
                io.qt[:, ch * 512:(ch + 1) * 512].rearrange(
                    "(kt p) s -> p kt s", p=128),
            )
        for ch in range(4):
            nc.sync.dma_start(
                xk[ch][:],
                io.kt[:, ch * 512:(ch + 1) * 512].rearrange(
                    "(kt p) s -> p kt s", p=128),
            )

        # v projection (all of it is needed before the first ctx matmul).
        for st_i in range(16):
            ps = pp.tile([128, JC], f32, tag="proj")
            for ktile in range(8):
                nc.tensor.matmul(
                    ps[:],
                    xv[st_i // 4][:, ktile, (st_i % 4) * 128:(st_i % 4 + 1) * 128],
                    wv_sb[:, ktile, :],
                    start=(ktile == 0),
                    stop=(ktile == 7),
                )
            nc.vector.tensor_add(
                sb.v_sb[:, st_i, :, 0:DK],
                ps[:].rearrange("p (h c) -> p h c", h=HC),
                sb.bvb_sb[:].rearrange("p (h c) -> p h c", h=HC),
            )

        def proj_chunk(which, jt, sc):
            """One [128,512] projection chunk: q or k, head-pair jt, seq chunk sc."""
            x_sb, w_sb, o_sb, b_sb = (
                (xq[sc], wq_sb, sb.qT_sb, sb.bq_sb) if which == "q"
                else (xk[sc], wk_sb, sb.kT_sb, sb.bk_sb))
            ps = pp.tile([128, 512], f32, tag="proj")
            for ktile in range(8):
                nc.tensor.matmul(
                    ps[:],
                    w_sb[:, ktile, jt * 128:(jt + 1) * 128],
                    x_sb[:, ktile, :],
                    start=(ktile == 0),
                    stop=(ktile == 7),
                )
            nc.vector.tensor_scalar_add(
                o_sb[:, jt, sc * 512:(sc + 1) * 512], ps[:], b_sb[:, jt:jt + 1])

        for sc in range(4):
            proj_chunk("q", 0, sc)
        for sc in range(4):
            proj_chunk("k", 0, sc)

        # Fillers: head-pair jt's 8 projection chunks spread 2-per-block over
        # the 4 blocks of head-pair jt-1 (at iterations 5 and 13).
        fillers = {}  # (block, k) -> (which, jt, sc)
        for jt in range(1, 4):
            chunks = [("q", jt, sc) for sc in range(4)] + \
                     [("k", jt, sc) for sc in range(4)]
            base = (jt - 1) * 4
            for i, c in enumerate(chunks):
                fillers[(base + i // 2, 5 if i % 2 == 0 else 13)] = c

        def emit_scores(h, sqb, k, st):
            lhs = sb.kT_sb[64 * (h % 2):64 * (h % 2) + 64,
                           h // 2, k * 128:(k + 1) * 128]
            q0 = sqb * 1024
            nc.tensor.matmul(
                st[:, 0:512], lhs,
                sb.qT_sb[64 * (h % 2):64 * (h % 2) + 64, h // 2, q0:q0 + 512],
                start=True, stop=True,
            )
            nc.tensor.matmul(
                st[:, 512:1024], lhs,
                sb.qT_sb[64 * (h % 2):64 * (h % 2) + 64, h // 2,
                         q0 + 512:q0 + 1024],
                start=True, stop=True,
            )

        ctxs = {}

        def emit_ctx(h, sqb, k, pt):
            c0, c1 = ctxs[(h, sqb)]
            vt = sb.v_sb[:, k, h, :]
            nc.tensor.matmul(c0[:], vt, pt[:, 0:512], start=(k == 0), stop=(k == 15))
            nc.tensor.matmul(c1[:], vt, pt[:, 512:1024], start=(k == 0), stop=(k == 15))
            if k == 15:
                jt = h // 2
                pbase = 64 * (h % 2)
                for ci, ctx in enumerate((c0, c1)):
                    sq = sqb * 2 + ci
                    # free the ctx psum bank with two quick copies, then run
                    # the normalization chain entirely from SBUF so the next
                    # block's ctx accumulation never waits on it.
                    lr = att2.tile([1, 512], f32, tag=f"l{ci}", name=f"l_{h}_{sq}")
                    nc.vector.tensor_copy(lr[:], ctx[DK:DK + 1, :])
                    cf = att2.tile([64, 512], f32, tag=f"cf{ci}", name=f"cf_{h}_{sq}")
                    nc.vector.tensor_copy(cf[:], ctx[0:DK, :])
                    r = att2.tile([1, 512], f32, tag=f"r{ci}", name=f"r_{h}_{sq}")
                    nc.vector.reciprocal_approx_fast(r[:], lr[:])
                    rb = att2.tile([64, 512], f32, tag=f"rb{ci}", name=f"rb_{h}_{sq}")
                    nc.gpsimd.partition_broadcast(rb[:], r[:])
                    nc.vector.tensor_mul(
                        sb.ctxn_sb[pbase:pbase + 64, jt, sq * 512:(sq + 1) * 512],
                        cf[:], rb[:],
                    )
                del ctxs[(h, sqb)]

        def out_proj(sq2, n):
            """One [128,512] output-projection group + copy-out + DMA."""
            ps = pp.tile([128, 512], f32, tag="proj")
            for p in range(4):
                nc.tensor.matmul(
                    ps[:],
                    sb.ctxn_sb[:, p, sq2 * 128:(sq2 + 1) * 128],
                    sb.wot_sb[:, p, n * 512:(n + 1) * 512],
                    start=(p == 0), stop=(p == 3),
                )
            ob = work.tile([128, 512], f32, tag="ob")
            nc.vector.tensor_copy(ob[:], ps[:])
            nc.sync.dma_start(
                io.out[sq2 * 128:(sq2 + 1) * 128, n * 512:(n + 1) * 512],
                ob[:],
            )

        # Flat depth-2 pipeline over all 256 (block, k) iterations:
        # iteration i emits scores(i), exp(i-1), ctx(i-2).
        iters = [(h, sqb, k) for (h, sqb) in BLOCKS for k in range(16)]
        exps = {}   # i -> (h, sqb, k, st)
        pts = {}    # i -> (h, sqb, k, pt)
        done_groups = set()
        for i, (h, sqb, k) in enumerate(iters):
            if k == 0:
                ctxs[(h, sqb)] = (
                    pp_ctx.tile([DK + 1, 512], f32, tag="ctx0",
                                name=f"ctx0_{h}_{sqb}"),
                    pp_ctx.tile([DK + 1, 512], f32, tag="ctx1",
                                name=f"ctx1_{h}_{sqb}"),
                )
            st = pp_st.tile([128, 1024], f32, tag="st")
            emit_scores(h, sqb, k, st)
            exps[i] = (h, sqb, k, st)
            if i - 1 in exps:
                eh, esqb, ek, est = exps.pop(i - 1)
                pt = att.tile([128, 1024], bf16, tag="pt")
                nc.scalar.activation(pt[:], est[:], EXP, scale=SCALE)
                pts[i - 1] = (eh, esqb, ek, pt)
            if i - 2 in pts:
                emit_ctx(*pts.pop(i - 2))
            f = fillers.get((i // 16, i % 16))
            if f is not None:
                proj_chunk(*f)
            # first half of the output projection (16 groups covering
            # s_q 0:1024) rides in the last 2 blocks, one group every 2
            # iterations, starting after block 13's normalization.
            if i >= 226 and (i - 226) % 2 == 0:
                g = (i - 226) // 2
                if g < 16:
                    out_proj(g // 2, g % 2)
                    done_groups.add(g)
        # drain the pipeline
        for i in (256, 257):
            if i - 1 in exps:
                eh, esqb, ek, est = exps.pop(i - 1)
                pt = att.tile([128, 1024], bf16, tag="pt")
                nc.scalar.activation(pt[:], est[:], EXP, scale=SCALE)
                pts[i - 1] = (eh, esqb, ek, pt)
            if i - 2 in pts:
                emit_ctx(*pts.pop(i - 2))
        # leftover first-half groups + second half of the output projection
        for g in range(16):
            if g not in done_groups:
                out_proj(g // 2, g % 2)
        for sq2 in range(8, 16):
            out_proj(sq2, 0)
            out_proj(sq2, 1)


class _NS:
    pass


def build_nc(repeats=1):
    nc = bacc.Bacc(None, target_bir_lowering=False, debug=False)

    io = _NS()
    io.qt = nc.dram_tensor("qt", [D, S], bf16, kind="ExternalInput")
    io.kt = nc.dram_tensor("kt", [D, S], bf16, kind="ExternalInput")
    io.vt = nc.dram_tensor("vt", [D, S], bf16, kind="ExternalInput")
    io.wqt = nc.dram_tensor("wqt", [D, JC], bf16, kind="ExternalInput")
    io.wkt = nc.dram_tensor("wkt", [D, JC], bf16, kind="ExternalInput")
    io.wvt = nc.dram_tensor("wvt", [D, JC], bf16, kind="ExternalInput")
    io.wot = nc.dram_tensor("wot", [JC, D], bf16, kind="ExternalInput")
    io.bq = nc.dram_tensor("bq", [128, 4], f32, kind="ExternalInput")
    io.bk = nc.dram_tensor("bk", [128, 4], f32, kind="ExternalInput")
    io.bvb = nc.dram_tensor("bvb", [128, JC], f32, kind="ExternalInput")
    io.out = nc.dram_tensor("out", [S, D], f32, kind="ExternalOutput")

    with tile.TileContext(nc) as tc:
        for _rep in range(repeats):
            with (
                tc.tile_pool(name="big", bufs=1) as big,
                tc.tile_pool(name="work", bufs=3) as work,
            ):
                sb = _NS()
                sb.qT_sb = big.tile([128, 4, S], bf16)           # [p, jt, s]
                sb.kT_sb = big.tile([128, 4, S], bf16)
                sb.v_sb = big.tile([128, 16, HC, DK + 1], bf16)  # [p, st, h, c]
                sb.bq_sb = big.tile([128, 4], f32)
                sb.bk_sb = big.tile([128, 4], f32)
                sb.bvb_sb = big.tile([128, JC], f32)

                nc.sync.dma_start(sb.bq_sb[:], io.bq[:])
                nc.sync.dma_start(sb.bk_sb[:], io.bk[:])
                nc.sync.dma_start(sb.bvb_sb[:], io.bvb[:])

                with tc.tile_pool(name="big2", bufs=1) as big2:
                    sb.ctxn_sb = big2.tile([128, 4, S], bf16)    # [p, pair, s]
                    sb.wot_sb = big2.tile([128, 4, D], bf16)
                    nc.sync.dma_start(
                        sb.wot_sb[:],
                        io.wot.rearrange("(kt p) j -> p kt j", p=128),
                    )
                    _stage12(nc, tc, work, io, sb)

    nc.compile()
    return nc


_NC = None


def _get_nc():
    global _NC
    if _NC is None:
        _NC = build_nc()
    return _NC


def make_in_maps(Q, K, V, Wq, bq, Wk, bk, Wv, bv, Wo, bo):
    asb = lambda x: np.ascontiguousarray(
        np.asarray(x, dtype=np.float32).astype(ml_dtypes.bfloat16))
    asf = lambda x: np.ascontiguousarray(np.asarray(x, dtype=np.float32))
    in_maps = []
    for c in range(N_CORES):
        b = c // 2
        j0 = JC * (c % 2)
        jsl = slice(j0, j0 + JC)
        in_maps.append({
            "qt": asb(np.asarray(Q)[b].T),
            "kt": asb(np.asarray(K)[b].T),
            "vt": asb(np.asarray(V)[b].T),
            "wqt": asb(np.asarray(Wq)[jsl].T),
            "wkt": asb(np.asarray(Wk)[jsl].T),
            "wvt": asb(np.asarray(Wv)[jsl].T),
            "wot": asb(np.asarray(Wo)[:, jsl].T),
            "bq": asf(np.asarray(bq)[jsl].reshape(4, 128).T),
            "bk": asf(np.asarray(bk)[jsl].reshape(4, 128).T),
            "bvb": asf(np.broadcast_to(np.asarray(bv)[jsl], (128, JC))),
        })
    return in_maps


def kernel(Q, K, V, Wq, bq, Wk, bk, Wv, bv, Wo, bo, _trace=False, _trace_kwargs=None):
    nc = _get_nc()
    in_maps = make_in_maps(Q, K, V, Wq, bq, Wk, bk, Wv, bv, Wo, bo)
    res = run_bass_kernel_spmd(
        nc, in_maps, core_ids=list(range(N_CORES)),
        trace=_trace, **(_trace_kwargs or {}),
    )
    parts = [res.results[c]["out"] for c in range(N_CORES)]
    bo_np = np.asarray(bo, dtype=np.float32)
    O = np.stack([parts[2 * b] + parts[2 * b + 1] + bo_np for b in range(4)])
    kernel.last_results = res
    return O.astype(np.float32)


# revision 18
# speedup vs baseline: 1.0992x; 1.0095x over previous
"""Multi-head attention (B=4, S=2048, D=1024, H=16) on 8 TRN2 NeuronCores.

Sharding: core c <- batch c//2, heads 8*(c%2) .. 8*(c%2)+8 (Megatron-style:
Wq/Wk/Wv column-parallel, Wo row-parallel). No collectives: the two partial
outputs per batch are summed on the host (plus the bo bias).

Per-core kernel strategy (v6 — bf16, flat depth-2 attention pipeline):
  - All matmul operands are bf16 (PSUM accumulation stays fp32). The PE
    moving-operand SBUF port supplies 2 B/lane/cycle at 2.4 GHz, so fp32
    streams are capped at an effective 1.2 GHz while bf16 streams run at
    full clock: bf16 halves every matmul's wall time.
  - q^T, k^T computed directly in [head_dim, seq] layout (out = W^T.T @ X^T),
    v computed in natural [seq, head_dim] layout with a ones column appended.
  - Scores computed transposed: ST[s_k, s_q] = k . q, so softmax exp is pure
    elementwise (no max subtraction needed: scores ~ N(0,1) after 1/8 scale)
    and no on-chip transposes are needed anywhere.
  - ctx^T[c, s_q] accumulated as v_aug^T @ exp(ST); the ones column yields the
    softmax denominator l[s_q] as psum row 64 for free.
  - Normalization by 1/l: copy the l row to partition 0 (reciprocal_approx_fast
    misreads partition-offset APs), approx-reciprocal, gpsimd broadcast, mul.
  - The 256 attention iterations run as ONE flat software pipeline with
    explicit depth 2: iteration i emits scores(i), exp(i-1), ctx(i-2). The
    exp's input is a full iteration old, so ACT never eats the ~100 ns
    cross-engine semaphore latency, and the ~300 ns/iter of PE slack
    accumulates into real lookahead.
  - Only v and the jt=0 q/k projections run before attention; remaining q/k
    projections drop into the pipeline as PE filler (2 chunks per 16-iter
    block), absorbed by the ACT shadow. Blocks run jt-major
    [(2jt,s0),(2jt+1,s0),(2jt,s1),(2jt+1,s1)] so every head's sqb=0 finishes
    by block 13 and the first half of the output projection overlaps the
    last two blocks. PSUM: st x2 (4 banks) + ctx0/ctx1 (2) + proj x2 (2) = 8.
"""
import sys

sys.path.insert(0, "/opt/trn_rl_repo")
import numpy as np
import ml_dtypes

import concourse.bass as bass
import concourse.bacc as bacc
import concourse.mybir as mybir
import concourse.tile as tile
from concourse.bass_utils import run_bass_kernel_spmd

f32 = mybir.dt.float32
bf16 = mybir.dt.bfloat16
EXP = mybir.ActivationFunctionType.Exp

S = 2048          # sequence length
D = 1024          # model dim
HC = 8            # heads per core
DK = 64           # head dim
JC = HC * DK      # per-core projection width (512)
SCALE = 0.125     # 1/sqrt(DK)
N_CORES = 8

# Block order: jt-major so head-pair jt's q/k projections are only needed
# 4 blocks after head-pair jt-1's blocks start, and all sqb=0 blocks are
# done by index 13.
BLOCKS = []
for _jt in range(4):
    BLOCKS += [(2 * _jt, 0), (2 * _jt + 1, 0), (2 * _jt, 1), (2 * _jt + 1, 1)]


def _stage12(nc, tc, work, io, sb):
    """v + jt0 q/k projections, then the flat attention pipeline with the
    remaining projections and the first half of the output projection
    interleaved as PE filler."""
    nc.vector.memset(sb.v_sb[:, :, :, DK], 1.0)
    with (
        tc.tile_pool(name="w1", bufs=1) as wp,
        tc.tile_pool(name="x1", bufs=1) as xp,
        tc.tile_pool(name="psproj", bufs=2, space="PSUM") as pp,
        tc.tile_pool(name="ps2st", bufs=2, space="PSUM") as pp_st,
        tc.tile_pool(name="ps2ctx", bufs=1, space="PSUM") as pp_ctx,
        tc.tile_pool(name="att", bufs=7) as att,
        tc.tile_pool(name="att2", bufs=1) as att2,
    ):
        wq_sb = wp.tile([128, 8, JC], bf16, tag="wq")
        wk_sb = wp.tile([128, 8, JC], bf16, tag="wk")
        wv_sb = wp.tile([128, 8, JC], bf16, tag="wv")
        # per-chunk x tiles so a projection chunk only waits on its own DMA
        xv = [xp.tile([128, 8, 512], bf16, tag=f"xvk{c}", name=f"xv{c}")
              for c in range(4)]
        xq = [xp.tile([128, 8, 512], bf16, tag=f"xq{c}", name=f"xq{c}")
              for c in range(4)]
        # k chunks reuse the v chunk slots (v is fully consumed before k DMAs)
        xk = [xp.tile([128, 8, 512], bf16, tag=f"xvk{c}", name=f"xk{c}")
              for c in range(4)]
        for hh in range(2):
            nc.sync.dma_start(
                wv_sb[:, 4 * hh:4 * hh + 4, :],
                io.wvt[512 * hh:512 * hh + 512].rearrange(
                    "(kt p) j -> p kt j", p=128))
        for ch in range(4):
            for hh in range(2):
                nc.sync.dma_start(
                    xv[ch][:, :, 256 * hh:256 * hh + 256],
                    io.vt[:, ch * 512 + 256 * hh:ch * 512 + 256 * hh + 256
                          ].rearrange("(kt p) s -> p kt s", p=128),
                )
        nc.sync.dma_start(wq_sb[:], io.wqt.rearrange("(kt p) j -> p kt j", p=128))
        nc.sync.dma_start(wk_sb[:], io.wkt.rearrange("(kt p) j -> p kt j", p=128))
        for ch in range(4):
            nc.sync.dma_start(
                xq[ch][:],
                io.qt[:, ch * 512:(ch + 1) * 512].rearrange(
                    "(kt p) s -> p kt s", p=128),
            )
        for ch in range(4):
            nc.sync.dma_start(
                xk[ch][:],
                io.kt[:, ch * 512:(ch + 1) * 512].rearrange(
                    "(kt p) s -> p kt s", p=128),
            )

        # v projection (all of it is needed before the first ctx matmul).
        for st_i in range(16):
            ps = pp.tile([128, JC], f32, tag="proj")
            for ktile in range(8):
                nc.tensor.matmul(
                    ps[:],
                    xv[st_i // 4][:, ktile, (st_i % 4) * 128:(st_i % 4 + 1) * 128],
                    wv_sb[:, ktile, :],
                    start=(ktile == 0),
                    stop=(ktile == 7),
                )
            nc.vector.tensor_add(
                sb.v_sb[:, st_i, :, 0:DK],
                ps[:].rearrange("p (h c) -> p h c", h=HC),
                sb.bvb_sb[:].rearrange("p (h c) -> p h c", h=HC),
            )

        def proj_chunk(which, jt, sc):
            """One [128,512] projection chunk: q or k, head-pair jt, seq chunk sc."""
            x_sb, w_sb, o_sb, b_sb = (
                (xq[sc], wq_sb, sb.qT_sb, sb.bq_sb) if which == "q"
                else (xk[sc], wk_sb, sb.kT_sb, sb.bk_sb))
            ps = pp.tile([128, 512], f32, tag="proj")
            for ktile in range(8):
                nc.tensor.matmul(
                    ps[:],
                    w_sb[:, ktile, jt * 128:(jt + 1) * 128],
                    x_sb[:, ktile, :],
                    start=(ktile == 0),
                    stop=(ktile == 7),
                )
            nc.vector.tensor_scalar_add(
                o_sb[:, jt, sc * 512:(sc + 1) * 512], ps[:], b_sb[:, jt:jt + 1])

        for sc in range(4):
            proj_chunk("q", 0, sc)
        for sc in range(4):
            proj_chunk("k", 0, sc)

        # Fillers: head-pair jt's 8 projection chunks spread 2-per-block over
        # the 4 blocks of head-pair jt-1 (at iterations 5 and 13).
        fillers = {}  # (block, k) -> (which, jt, sc)
        for jt in range(1, 4):
            chunks = [("q", jt, sc) for sc in range(4)] + \
                     [("k", jt, sc) for sc in range(4)]
            base = (jt - 1) * 4
            for i, c in enumerate(chunks):
                fillers[(base + i // 2, 5 if i % 2 == 0 else 13)] = c

        def emit_scores(h, sqb, k, st):
            lhs = sb.kT_sb[64 * (h % 2):64 * (h % 2) + 64,
                           h // 2, k * 128:(k + 1) * 128]
            q0 = sqb * 1024
            nc.tensor.matmul(
                st[:, 0:512], lhs,
                sb.qT_sb[64 * (h % 2):64 * (h % 2) + 64, h // 2, q0:q0 + 512],
                start=True, stop=True,
            )
            nc.tensor.matmul(
                st[:, 512:1024], lhs,
                sb.qT_sb[64 * (h % 2):64 * (h % 2) + 64, h // 2,
                         q0 + 512:q0 + 1024],
                start=True, stop=True,
            )

        ctxs = {}

        def emit_ctx(h, sqb, k, pt):
            c0, c1 = ctxs[(h, sqb)]
            vt = sb.v_sb[:, k, h, :]
            nc.tensor.matmul(c0[:], vt, pt[:, 0:512], start=(k == 0), stop=(k == 15))
            nc.tensor.matmul(c1[:], vt, pt[:, 512:1024], start=(k == 0), stop=(k == 15))
            if k == 15:
                jt = h // 2
                pbase = 64 * (h % 2)
                for ci, ctx in enumerate((c0, c1)):
                    sq = sqb * 2 + ci
                    # free the ctx psum bank with two quick copies, then run
                    # the normalization chain entirely from SBUF so the next
                    # block's ctx accumulation never waits on it.
                    lr = att2.tile([1, 512], f32, tag=f"l{ci}", name=f"l_{h}_{sq}")
                    nc.vector.tensor_copy(lr[:], ctx[DK:DK + 1, :])
                    cf = att2.tile([64, 512], f32, tag=f"cf{ci}", name=f"cf_{h}_{sq}")
                    nc.vector.tensor_copy(cf[:], ctx[0:DK, :])
                    r = att2.tile([1, 512], f32, tag=f"r{ci}", name=f"r_{h}_{sq}")
                    nc.vector.reciprocal_approx_fast(r[:], lr[:])
                    rb = att2.tile([64, 512], f32, tag=f"rb{ci}", name=f"rb_{h}_{sq}")
                    nc.gpsimd.partition_broadcast(rb[:], r[:])
                    nc.vector.tensor_mul(
                        sb.ctxn_sb[pbase:pbase + 64, jt, sq * 512:(sq + 1) * 512],
                        cf[:], rb[:],
                    )
                del ctxs[(h, sqb)]

        def out_proj(sq2, n, on_act=False):
            """One [128,512] output-projection group + copy-out + DMA."""
            ps = pp.tile([128, 512], f32, tag="proj")
            for p in range(4):
                nc.tensor.matmul(
                    ps[:],
                    sb.ctxn_sb[:, p, sq2 * 128:(sq2 + 1) * 128],
                    sb.wot_sb[:, p, n * 512:(n + 1) * 512],
                    start=(p == 0), stop=(p == 3),
                )
            ob = work.tile([128, 512], bf16, tag="ob")
            nc.vector.tensor_copy(ob[:], ps[:])
            nc.sync.dma_start(
                io.out[sq2 * 128:(sq2 + 1) * 128, n * 512:(n + 1) * 512],
                ob[:],
            )

        # Flat depth-2 pipeline over all 256 (block, k) iterations:
        # iteration i emits scores(i), exp(i-1), ctx(i-2).
        iters = [(h, sqb, k) for (h, sqb) in BLOCKS for k in range(16)]
        exps = {}   # i -> (h, sqb, k, st)
        pts = {}    # i -> (h, sqb, k, pt)
        done_groups = set()
        for i, (h, sqb, k) in enumerate(iters):
            if k == 0:
                ctxs[(h, sqb)] = (
                    pp_ctx.tile([DK + 1, 512], f32, tag="ctx0",
                                name=f"ctx0_{h}_{sqb}"),
                    pp_ctx.tile([DK + 1, 512], f32, tag="ctx1",
                                name=f"ctx1_{h}_{sqb}"),
                )
            st = pp_st.tile([128, 1024], f32, tag="st")
            emit_scores(h, sqb, k, st)
            exps[i] = (h, sqb, k, st)
            if i - 1 in exps:
                eh, esqb, ek, est = exps.pop(i - 1)
                pt = att.tile([128, 1024], bf16, tag="pt")
                nc.scalar.activation(pt[:], est[:], EXP, scale=SCALE)
                pts[i - 1] = (eh, esqb, ek, pt)
            if i - 2 in pts:
                emit_ctx(*pts.pop(i - 2))
            f = fillers.get((i // 16, i % 16))
            if f is not None:
                proj_chunk(*f)
            # first half of the output projection (16 groups covering
            # s_q 0:1024) rides in the last 2 blocks, one group every 2
            # iterations, starting after block 13's normalization.
            # NOTE: block 13's normalization (last sqb=0 ctxn write) is
            # emitted at i=228 with the lag-5 ctx schedule; out_proj reads
            # must be emitted after it or they pick up stale ctxn.
            if i >= 230 and (i - 230) % 2 == 0:
                g = (i - 230) // 2
                if g < 16:
                    out_proj(g // 2, g % 2)
                    done_groups.add(g)
        # drain the pipeline
        for i in (256, 257):
            if i - 1 in exps:
                eh, esqb, ek, est = exps.pop(i - 1)
                pt = att.tile([128, 1024], bf16, tag="pt")
                nc.scalar.activation(pt[:], est[:], EXP, scale=SCALE)
                pts[i - 1] = (eh, esqb, ek, pt)
            if i - 2 in pts:
                emit_ctx(*pts.pop(i - 2))
        # leftover first-half groups + second half of the output projection
        for g in range(16):
            if g not in done_groups:
                out_proj(g // 2, g % 2, on_act=True)
        for sq2 in range(8, 16):
            out_proj(sq2, 0, on_act=True)
            out_proj(sq2, 1, on_act=True)


class _NS:
    pass


def build_nc(repeats=1):
    nc = bacc.Bacc(None, target_bir_lowering=False, debug=False)

    io = _NS()
    io.qt = nc.dram_tensor("qt", [D, S], bf16, kind="ExternalInput")
    io.kt = nc.dram_tensor("kt", [D, S], bf16, kind="ExternalInput")
    io.vt = nc.dram_tensor("vt", [D, S], bf16, kind="ExternalInput")
    io.wqt = nc.dram_tensor("wqt", [D, JC], bf16, kind="ExternalInput")
    io.wkt = nc.dram_tensor("wkt", [D, JC], bf16, kind="ExternalInput")
    io.wvt = nc.dram_tensor("wvt", [D, JC], bf16, kind="ExternalInput")
    io.wot = nc.dram_tensor("wot", [JC, D], bf16, kind="ExternalInput")
    io.bq = nc.dram_tensor("bq", [128, 4], f32, kind="ExternalInput")
    io.bk = nc.dram_tensor("bk", [128, 4], f32, kind="ExternalInput")
    io.bvb = nc.dram_tensor("bvb", [128, JC], f32, kind="ExternalInput")
    io.out = nc.dram_tensor("out", [S, D], bf16, kind="ExternalOutput")

    with tile.TileContext(nc) as tc:
        for _rep in range(repeats):
            with (
                tc.tile_pool(name="big", bufs=1) as big,
                tc.tile_pool(name="work", bufs=3) as work,
            ):
                sb = _NS()
                sb.qT_sb = big.tile([128, 4, S], bf16)           # [p, jt, s]
                sb.kT_sb = big.tile([128, 4, S], bf16)
                sb.v_sb = big.tile([128, 16, HC, DK + 1], bf16)  # [p, st, h, c]
                sb.bq_sb = big.tile([128, 4], f32)
                sb.bk_sb = big.tile([128, 4], f32)
                sb.bvb_sb = big.tile([128, JC], f32)

                nc.sync.dma_start(sb.bq_sb[:], io.bq[:])
                nc.sync.dma_start(sb.bk_sb[:], io.bk[:])
                nc.sync.dma_start(sb.bvb_sb[:], io.bvb[:])

                with tc.tile_pool(name="big2", bufs=1) as big2:
                    sb.ctxn_sb = big2.tile([128, 4, S], bf16)    # [p, pair, s]
                    sb.wot_sb = big2.tile([128, 4, D], bf16)
                    nc.sync.dma_start(
                        sb.wot_sb[:],
                        io.wot.rearrange("(kt p) j -> p kt j", p=128),
                    )
                    _stage12(nc, tc, work, io, sb)

    nc.compile()
    return nc


_NC = None


def _get_nc():
    global _NC
    if _NC is None:
        _NC = build_nc()
    return _NC


def make_in_maps(Q, K, V, Wq, bq, Wk, bk, Wv, bv, Wo, bo):
    asb = lambda x: np.ascontiguousarray(
        np.asarray(x, dtype=np.float32).astype(ml_dtypes.bfloat16))
    asf = lambda x: np.ascontiguousarray(np.asarray(x, dtype=np.float32))
    in_maps = []
    for c in range(N_CORES):
        b = c // 2
        j0 = JC * (c % 2)
        jsl = slice(j0, j0 + JC)
        in_maps.append({
            "qt": asb(np.asarray(Q)[b].T),
            "kt": asb(np.asarray(K)[b].T),
            "vt": asb(np.asarray(V)[b].T),
            "wqt": asb(np.asarray(Wq)[jsl].T),
            "wkt": asb(np.asarray(Wk)[jsl].T),
            "wvt": asb(np.asarray(Wv)[jsl].T),
            "wot": asb(np.asarray(Wo)[:, jsl].T),
            "bq": asf(np.asarray(bq)[jsl].reshape(4, 128).T),
            "bk": asf(np.asarray(bk)[jsl].reshape(4, 128).T),
            "bvb": asf(np.broadcast_to(np.asarray(bv)[jsl], (128, JC))),
        })
    return in_maps


def kernel(Q, K, V, Wq, bq, Wk, bk, Wv, bv, Wo, bo, _trace=False, _trace_kwargs=None):
    nc = _get_nc()
    in_maps = make_in_maps(Q, K, V, Wq, bq, Wk, bk, Wv, bv, Wo, bo)
    res = run_bass_kernel_spmd(
        nc, in_maps, core_ids=list(range(N_CORES)),
        trace=_trace, **(_trace_kwargs or {}),
    )
    parts = [res.results[c]["out"].astype(np.float32) for c in range(N_CORES)]
    bo_np = np.asarray(bo, dtype=np.float32)
    O = np.stack([parts[2 * b] + parts[2 * b + 1] + bo_np for b in range(4)])
    kernel.last_results = res
    return O.astype(np.float32)


# revision 19
# speedup vs baseline: 1.1063x; 1.0064x over previous
"""Multi-head attention (B=4, S=2048, D=1024, H=16) on 8 TRN2 NeuronCores.

Sharding: core c <- batch c//2, heads 8*(c%2) .. 8*(c%2)+8 (Megatron-style:
Wq/Wk/Wv column-parallel, Wo row-parallel). No collectives: the two partial
outputs per batch are summed on the host (plus the bo bias).

Per-core kernel strategy (v6 — bf16, flat depth-2 attention pipeline):
  - All matmul operands are bf16 (PSUM accumulation stays fp32). The PE
    moving-operand SBUF port supplies 2 B/lane/cycle at 2.4 GHz, so fp32
    streams are capped at an effective 1.2 GHz while bf16 streams run at
    full clock: bf16 halves every matmul's wall time.
  - q^T, k^T computed directly in [head_dim, seq] layout (out = W^T.T @ X^T),
    v computed in natural [seq, head_dim] layout with a ones column appended.
  - Scores computed transposed: ST[s_k, s_q] = k . q, so softmax exp is pure
    elementwise (no max subtraction needed: scores ~ N(0,1) after 1/8 scale)
    and no on-chip transposes are needed anywhere.
  - ctx^T[c, s_q] accumulated as v_aug^T @ exp(ST); the ones column yields the
    softmax denominator l[s_q] as psum row 64 for free.
  - Normalization by 1/l: copy the l row to partition 0 (reciprocal_approx_fast
    misreads partition-offset APs), approx-reciprocal, gpsimd broadcast, mul.
  - The 256 attention iterations run as ONE flat software pipeline with
    explicit depth 2: iteration i emits scores(i), exp(i-1), ctx(i-2). The
    exp's input is a full iteration old, so ACT never eats the ~100 ns
    cross-engine semaphore latency, and the ~300 ns/iter of PE slack
    accumulates into real lookahead.
  - Only v and the jt=0 q/k projections run before attention; remaining q/k
    projections drop into the pipeline as PE filler (2 chunks per 16-iter
    block), absorbed by the ACT shadow. Blocks run jt-major
    [(2jt,s0),(2jt+1,s0),(2jt,s1),(2jt+1,s1)] so every head's sqb=0 finishes
    by block 13 and the first half of the output projection overlaps the
    last two blocks. PSUM: st x2 (4 banks) + ctx0/ctx1 (2) + proj x2 (2) = 8.
"""
import sys

sys.path.insert(0, "/opt/trn_rl_repo")
import numpy as np
import ml_dtypes

import concourse.bass as bass
import concourse.bacc as bacc
import concourse.mybir as mybir
import concourse.tile as tile
from concourse.bass_utils import run_bass_kernel_spmd

f32 = mybir.dt.float32
bf16 = mybir.dt.bfloat16
EXP = mybir.ActivationFunctionType.Exp

S = 2048          # sequence length
D = 1024          # model dim
HC = 8            # heads per core
DK = 64           # head dim
JC = HC * DK      # per-core projection width (512)
SCALE = 0.125     # 1/sqrt(DK)
N_CORES = 8

# Block order: jt-major so head-pair jt's q/k projections are only needed
# 4 blocks after head-pair jt-1's blocks start, and all sqb=0 blocks are
# done by index 13.
BLOCKS = []
for _jt in range(4):
    BLOCKS += [(2 * _jt, 0), (2 * _jt + 1, 0), (2 * _jt, 1), (2 * _jt + 1, 1)]


def _stage12(nc, tc, work, io, sb):
    """v + jt0 q/k projections, then the flat attention pipeline with the
    remaining projections and the first half of the output projection
    interleaved as PE filler."""
    nc.vector.memset(sb.v_sb[:, :, :, DK], 1.0)
    with (
        tc.tile_pool(name="w1", bufs=1) as wp,
        tc.tile_pool(name="x1", bufs=1) as xp,
        tc.tile_pool(name="psproj", bufs=2, space="PSUM") as pp,
        tc.tile_pool(name="ps2st", bufs=2, space="PSUM") as pp_st,
        tc.tile_pool(name="ps2ctx", bufs=1, space="PSUM") as pp_ctx,
        tc.tile_pool(name="att", bufs=7) as att,
        tc.tile_pool(name="att2", bufs=1) as att2,
    ):
        wq_sb = wp.tile([128, 8, JC], bf16, tag="wq")
        wk_sb = wp.tile([128, 8, JC], bf16, tag="wk")
        wv_sb = wp.tile([128, 8, JC], bf16, tag="wv")
        # per-chunk x tiles so a projection chunk only waits on its own DMA
        xv = [xp.tile([128, 8, 512], bf16, tag=f"xvk{c}", name=f"xv{c}")
              for c in range(4)]
        xq = [xp.tile([128, 8, 512], bf16, tag=f"xq{c}", name=f"xq{c}")
              for c in range(4)]
        # k chunks reuse the v chunk slots (v is fully consumed before k DMAs)
        xk = [xp.tile([128, 8, 512], bf16, tag=f"xvk{c}", name=f"xk{c}")
              for c in range(4)]
        for hh in range(2):
            nc.sync.dma_start(
                wv_sb[:, 4 * hh:4 * hh + 4, :],
                io.wvt[512 * hh:512 * hh + 512].rearrange(
                    "(kt p) j -> p kt j", p=128))
        for ch in range(4):
            for hh in range(2):
                nc.sync.dma_start(
                    xv[ch][:, :, 256 * hh:256 * hh + 256],
                    io.vt[:, ch * 512 + 256 * hh:ch * 512 + 256 * hh + 256
                          ].rearrange("(kt p) s -> p kt s", p=128),
                )
        nc.sync.dma_start(wq_sb[:], io.wqt.rearrange("(kt p) j -> p kt j", p=128))
        nc.sync.dma_start(wk_sb[:], io.wkt.rearrange("(kt p) j -> p kt j", p=128))
        for ch in range(4):
            nc.sync.dma_start(
                xq[ch][:],
                io.qt[:, ch * 512:(ch + 1) * 512].rearrange(
                    "(kt p) s -> p kt s", p=128),
            )
        for ch in range(4):
            nc.sync.dma_start(
                xk[ch][:],
                io.kt[:, ch * 512:(ch + 1) * 512].rearrange(
                    "(kt p) s -> p kt s", p=128),
            )

        # v projection (all of it is needed before the first ctx matmul).
        for st_i in range(16):
            ps = pp.tile([128, JC], f32, tag="proj")
            for ktile in range(8):
                nc.tensor.matmul(
                    ps[:],
                    xv[st_i // 4][:, ktile, (st_i % 4) * 128:(st_i % 4 + 1) * 128],
                    wv_sb[:, ktile, :],
                    start=(ktile == 0),
                    stop=(ktile == 7),
                )
            nc.vector.tensor_add(
                sb.v_sb[:, st_i, :, 0:DK],
                ps[:].rearrange("p (h c) -> p h c", h=HC),
                sb.bvb_sb[:].rearrange("p (h c) -> p h c", h=HC),
            )

        def proj_chunk(which, jt, sc):
            """One [128,512] projection chunk: q or k, head-pair jt, seq chunk sc."""
            x_sb, w_sb, o_sb, b_sb = (
                (xq[sc], wq_sb, sb.qT_sb, sb.bq_sb) if which == "q"
                else (xk[sc], wk_sb, sb.kT_sb, sb.bk_sb))
            ps = pp.tile([128, 512], f32, tag="proj")
            for ktile in range(8):
                nc.tensor.matmul(
                    ps[:],
                    w_sb[:, ktile, jt * 128:(jt + 1) * 128],
                    x_sb[:, ktile, :],
                    start=(ktile == 0),
                    stop=(ktile == 7),
                )
            nc.vector.tensor_scalar_add(
                o_sb[:, jt, sc * 512:(sc + 1) * 512], ps[:], b_sb[:, jt:jt + 1])

        for sc in range(4):
            proj_chunk("q", 0, sc)
        for sc in range(4):
            proj_chunk("k", 0, sc)

        # Fillers: head-pair jt's 8 projection chunks spread 2-per-block over
        # the 4 blocks of head-pair jt-1 (at iterations 5 and 13).
        fillers = {}  # (block, k) -> (which, jt, sc)
        for jt in range(1, 4):
            chunks = [("q", jt, sc) for sc in range(4)] + \
                     [("k", jt, sc) for sc in range(4)]
            base = (jt - 1) * 4
            for i, c in enumerate(chunks):
                fillers[(base + i // 2, 5 if i % 2 == 0 else 13)] = c

        def emit_scores(h, sqb, k, st):
            lhs = sb.kT_sb[64 * (h % 2):64 * (h % 2) + 64,
                           h // 2, k * 128:(k + 1) * 128]
            q0 = sqb * 1024
            nc.tensor.matmul(
                st[:, 0:512], lhs,
                sb.qT_sb[64 * (h % 2):64 * (h % 2) + 64, h // 2, q0:q0 + 512],
                start=True, stop=True,
            )
            nc.tensor.matmul(
                st[:, 512:1024], lhs,
                sb.qT_sb[64 * (h % 2):64 * (h % 2) + 64, h // 2,
                         q0 + 512:q0 + 1024],
                start=True, stop=True,
            )

        ctxs = {}

        def emit_ctx(h, sqb, k, pt):
            c0, c1 = ctxs[(h, sqb)]
            vt = sb.v_sb[:, k, h, :]
            nc.tensor.matmul(c0[:], vt, pt[:, 0:512], start=(k == 0), stop=(k == 15))
            nc.tensor.matmul(c1[:], vt, pt[:, 512:1024], start=(k == 0), stop=(k == 15))
            if k == 15:
                jt = h // 2
                pbase = 64 * (h % 2)
                for ci, ctx in enumerate((c0, c1)):
                    sq = sqb * 2 + ci
                    # free the ctx psum bank with ONE full-tile copy, then run
                    # the normalization chain entirely from SBUF so the next
                    # block's ctx accumulation only waits ~0.7us.
                    cf = att2.tile([DK + 1, 512], f32, tag=f"cf{ci}", name=f"cf_{h}_{sq}")
                    nc.vector.tensor_copy(cf[:], ctx[:])
                    # reciprocal_approx_fast misreads partition-offset APs, so
                    # stage the denominator row at partition 0 first.
                    lr = att2.tile([1, 512], f32, tag=f"l{ci}", name=f"l_{h}_{sq}")
                    nc.vector.tensor_copy(lr[:], cf[DK:DK + 1, :])
                    r = att2.tile([1, 512], f32, tag=f"r{ci}", name=f"r_{h}_{sq}")
                    nc.vector.reciprocal_approx_fast(r[:], lr[:])
                    rb = att2.tile([64, 512], f32, tag=f"rb{ci}", name=f"rb_{h}_{sq}")
                    nc.gpsimd.partition_broadcast(rb[:], r[:])
                    nc.vector.tensor_mul(
                        sb.ctxn_sb[pbase:pbase + 64, jt, sq * 512:(sq + 1) * 512],
                        cf[0:DK, :], rb[:],
                    )
                del ctxs[(h, sqb)]

        def out_proj(sq2, n, on_act=False):
            """One [128,512] output-projection group + copy-out + DMA."""
            ps = pp.tile([128, 512], f32, tag="proj")
            for p in range(4):
                nc.tensor.matmul(
                    ps[:],
                    sb.ctxn_sb[:, p, sq2 * 128:(sq2 + 1) * 128],
                    sb.wot_sb[:, p, n * 512:(n + 1) * 512],
                    start=(p == 0), stop=(p == 3),
                )
            ob = work.tile([128, 512], bf16, tag="ob")
            nc.vector.tensor_copy(ob[:], ps[:])
            nc.sync.dma_start(
                io.out[sq2 * 128:(sq2 + 1) * 128, n * 512:(n + 1) * 512],
                ob[:],
            )

        # Flat depth-2 pipeline over all 256 (block, k) iterations:
        # iteration i emits scores(i), exp(i-1), ctx(i-2).
        iters = [(h, sqb, k) for (h, sqb) in BLOCKS for k in range(16)]
        exps = {}   # i -> (h, sqb, k, st)
        pts = {}    # i -> (h, sqb, k, pt)
        done_groups = set()
        for i, (h, sqb, k) in enumerate(iters):
            if k == 0:
                ctxs[(h, sqb)] = (
                    pp_ctx.tile([DK + 1, 512], f32, tag="ctx0",
                                name=f"ctx0_{h}_{sqb}"),
                    pp_ctx.tile([DK + 1, 512], f32, tag="ctx1",
                                name=f"ctx1_{h}_{sqb}"),
                )
            st = pp_st.tile([128, 1024], f32, tag="st")
            emit_scores(h, sqb, k, st)
            exps[i] = (h, sqb, k, st)
            if i - 1 in exps:
                eh, esqb, ek, est = exps.pop(i - 1)
                pt = att.tile([128, 1024], bf16, tag="pt")
                nc.scalar.activation(pt[:], est[:], EXP, scale=SCALE)
                pts[i - 1] = (eh, esqb, ek, pt)
            if i - 2 in pts:
                emit_ctx(*pts.pop(i - 2))
            f = fillers.get((i // 16, i % 16))
            if f is not None:
                proj_chunk(*f)
            # first half of the output projection (16 groups covering
            # s_q 0:1024) rides in the last 2 blocks, one group every 2
            # iterations, starting after block 13's normalization.
            # NOTE: block 13's normalization (last sqb=0 ctxn write) is
            # emitted at i=228 with the lag-5 ctx schedule; out_proj reads
            # must be emitted after it or they pick up stale ctxn.
            if i >= 230 and (i - 230) % 2 == 0:
                g = (i - 230) // 2
                if g < 16:
                    out_proj(g // 2, g % 2)
                    done_groups.add(g)
        # drain the pipeline
        for i in (256, 257):
            if i - 1 in exps:
                eh, esqb, ek, est = exps.pop(i - 1)
                pt = att.tile([128, 1024], bf16, tag="pt")
                nc.scalar.activation(pt[:], est[:], EXP, scale=SCALE)
                pts[i - 1] = (eh, esqb, ek, pt)
            if i - 2 in pts:
                emit_ctx(*pts.pop(i - 2))
        # leftover first-half groups + second half of the output projection
        for g in range(16):
            if g not in done_groups:
                out_proj(g // 2, g % 2, on_act=True)
        for sq2 in range(8, 16):
            out_proj(sq2, 0, on_act=True)
            out_proj(sq2, 1, on_act=True)


class _NS:
    pass


def build_nc(repeats=1):
    nc = bacc.Bacc(None, target_bir_lowering=False, debug=False)

    io = _NS()
    io.qt = nc.dram_tensor("qt", [D, S], bf16, kind="ExternalInput")
    io.kt = nc.dram_tensor("kt", [D, S], bf16, kind="ExternalInput")
    io.vt = nc.dram_tensor("vt", [D, S], bf16, kind="ExternalInput")
    io.wqt = nc.dram_tensor("wqt", [D, JC], bf16, kind="ExternalInput")
    io.wkt = nc.dram_tensor("wkt", [D, JC], bf16, kind="ExternalInput")
    io.wvt = nc.dram_tensor("wvt", [D, JC], bf16, kind="ExternalInput")
    io.wot = nc.dram_tensor("wot", [JC, D], bf16, kind="ExternalInput")
    io.bq = nc.dram_tensor("bq", [128, 4], f32, kind="ExternalInput")
    io.bk = nc.dram_tensor("bk", [128, 4], f32, kind="ExternalInput")
    io.bvb = nc.dram_tensor("bvb", [128, JC], f32, kind="ExternalInput")
    io.out = nc.dram_tensor("out", [S, D], bf16, kind="ExternalOutput")

    with tile.TileContext(nc) as tc:
        for _rep in range(repeats):
            with (
                tc.tile_pool(name="big", bufs=1) as big,
                tc.tile_pool(name="work", bufs=3) as work,
            ):
                sb = _NS()
                sb.qT_sb = big.tile([128, 4, S], bf16)           # [p, jt, s]
                sb.kT_sb = big.tile([128, 4, S], bf16)
                sb.v_sb = big.tile([128, 16, HC, DK + 1], bf16)  # [p, st, h, c]
                sb.bq_sb = big.tile([128, 4], f32)
                sb.bk_sb = big.tile([128, 4], f32)
                sb.bvb_sb = big.tile([128, JC], f32)

                nc.sync.dma_start(sb.bq_sb[:], io.bq[:])
                nc.sync.dma_start(sb.bk_sb[:], io.bk[:])
                nc.sync.dma_start(sb.bvb_sb[:], io.bvb[:])

                with tc.tile_pool(name="big2", bufs=1) as big2:
                    sb.ctxn_sb = big2.tile([128, 4, S], bf16)    # [p, pair, s]
                    sb.wot_sb = big2.tile([128, 4, D], bf16)
                    nc.sync.dma_start(
                        sb.wot_sb[:],
                        io.wot.rearrange("(kt p) j -> p kt j", p=128),
                    )
                    _stage12(nc, tc, work, io, sb)

    nc.compile()
    return nc


_NC = None


def _get_nc():
    global _NC
    if _NC is None:
        _NC = build_nc()
    return _NC


def make_in_maps(Q, K, V, Wq, bq, Wk, bk, Wv, bv, Wo, bo):
    asb = lambda x: np.ascontiguousarray(
        np.asarray(x, dtype=np.float32).astype(ml_dtypes.bfloat16))
    asf = lambda x: np.ascontiguousarray(np.asarray(x, dtype=np.float32))
    in_maps = []
    for c in range(N_CORES):
        b = c // 2
        j0 = JC * (c % 2)
        jsl = slice(j0, j0 + JC)
        in_maps.append({
            "qt": asb(np.asarray(Q)[b].T),
            "kt": asb(np.asarray(K)[b].T),
            "vt": asb(np.asarray(V)[b].T),
            "wqt": asb(np.asarray(Wq)[jsl].T),
            "wkt": asb(np.asarray(Wk)[jsl].T),
            "wvt": asb(np.asarray(Wv)[jsl].T),
            "wot": asb(np.asarray(Wo)[:, jsl].T),
            "bq": asf(np.asarray(bq)[jsl].reshape(4, 128).T),
            "bk": asf(np.asarray(bk)[jsl].reshape(4, 128).T),
            "bvb": asf(np.broadcast_to(np.asarray(bv)[jsl], (128, JC))),
        })
    return in_maps


def kernel(Q, K, V, Wq, bq, Wk, bk, Wv, bv, Wo, bo, _trace=False, _trace_kwargs=None):
    nc = _get_nc()
    in_maps = make_in_maps(Q, K, V, Wq, bq, Wk, bk, Wv, bv, Wo, bo)
    res = run_bass_kernel_spmd(
        nc, in_maps, core_ids=list(range(N_CORES)),
        trace=_trace, **(_trace_kwargs or {}),
    )
    parts = [res.results[c]["out"].astype(np.float32) for c in range(N_CORES)]
    bo_np = np.asarray(bo, dtype=np.float32)
    O = np.stack([parts[2 * b] + parts[2 * b + 1] + bo_np for b in range(4)])
    kernel.last_results = res
    return O.astype(np.float32)
